# revision 30
# baseline (speedup 1.0000x reference)
"""GroupedRouter Bass kernel for 8 TRN2 NeuronCores.

Reference computation (per batch b, head h):
    q = x @ Wq, k = x @ Wk           (heads of dim 128)
    scores = q k^T / sqrt(128)       [N, N]
    group max over 8 key groups of 128, keep top-2 groups, softmax over kept.

Sharding: core c -> batch b = c//2, head half hh = c%2 (8 heads per core).
Fully data-parallel, no collectives.

Precision: every matmul = one fp16 main pass (fp16 products are exact in the
PE's fp22 pipeline) + one fp8(e4m3) DoubleRow correction pass at 0.5
cycles/row that carries the two first-order residual terms
(x2@W1 + x1@W2) with per-block power-of-2 scales chosen so all products land
at one PSUM scale. End-to-end score error ~6e-5, well inside the 2e-2 gate.
Projection residuals are prepared host-side; score-side residuals
(q2 = q - fp16(q)) are built on chip with a diag(-1) matmul into the proj
PSUM plus a second copyback.

Scale map (power-of-2, folded into copyback/exp scales):
  proj PSUM  = 2^15 * q     main: fp16(x*2^7) . fp16(W*2^8)
                            corr: e4(x2*2^10).e4(W1*2^5) + e4(x1*2^1).e4(W2*2^14)
  q1(fp16)   = 2^6  * q     copyback scale 2^-9
  qcat(e4m3) = [2^10 * q2 ; 2^2 * q1]   (copyback scale 2^-5; DVE scale 2^-4)
  score PSUM = 2^12 * s     main q1.k1, corr qcat.kcat (DoubleRow)
  exp        = exp(2^-12 * PSUM - 12)   (constant shift cancels in softmax)

Engines: PE matmuls ~210us; DVE group-max reduce + top-2 + per-group
mask/sum strips + normalize ~190us; ACT exp + sign ops + copybacks ~130us;
DMA fp16/fp8 in, bf16 out ~110us.
"""
import numpy as np
import orjson
import ml_dtypes

import concourse.bass as bass
import concourse.mybir as mybir
from concourse.tile import TileContext
from concourse.bass_utils import run_bass_kernel_spmd
from concourse.bass import ts, ds

B, N, D = 4, 1024, 2048
H, DH = 16, 128
G = 8
GSIZE = N // G          # 128
NCORES = 8
HPC = H // 2            # heads per core
SCALE = float(1.0 / np.sqrt(DH))

f32 = mybir.dt.float32
f16 = mybir.dt.float16
bf16 = mybir.dt.bfloat16
e4 = mybir.dt.float8e4
Alu = mybir.AluOpType
Act = mybir.ActivationFunctionType
AxX = mybir.AxisListType.X
DR = mybir.MatmulPerfMode.DoubleRow

# ---------------------------------------------------------------------------
# BIR sync-wait legalizer: walrus for cayman accepts only one sync-wait
# command per instruction; Tile attaches one per dependency. Hoist the excess
# onto standalone EventSemaphore instructions immediately before the target
# (engine queues are FIFO, so blocking semantics are unchanged).
# ---------------------------------------------------------------------------

def _legalize_bir(bir: dict) -> dict:
    ctr = 0
    for fn in bir["functions"]:
        for bb in fn["blocks"]:
            insts = bb.get("instructions")
            if not insts:
                continue
            out = []
            for ins in insts:
                si = ins.get("sync_info")
                waits = (si or {}).get("on_wait") or []
                if len(waits) > 1:
                    for w in waits[:-1]:
                        ctr += 1
                        out.append({
                            "engine": ins["engine"],
                            "ins": [],
                            "outs": [],
                            "name": f"legwait-{ctr}",
                            "opcode": "EventSemaphore",
                            "sync_info": {"on_update": [], "on_wait": [w]},
                        })
                    si["on_wait"] = waits[-1:]
                out.append(ins)
            bb["instructions"] = out
    return bir


def _install_legalizer(nc):
    orig = nc.to_json_bytes

    def to_json_bytes():
        return orjson.dumps(_legalize_bir(orjson.loads(orig())))

    nc.to_json_bytes = to_json_bytes


# ---------------------------------------------------------------------------
# Kernel build (one SPMD program; per-core differences live in the input data)
# ---------------------------------------------------------------------------

def _build():
    nc = bass.Bass()
    xm = nc.declare_dram_parameter("xm", [D, N], f16, isOutput=False)
    xc = nc.declare_dram_parameter("xc", [D, 2, N], e4, isOutput=False)
    wqm = nc.declare_dram_parameter("wqm", [D, HPC * DH], f16, isOutput=False)
    wkm = nc.declare_dram_parameter("wkm", [D, HPC * DH], f16, isOutput=False)
    # W residual pairs in per-head-major layout so each head's slice is a
    # balanced 4D DMA
    wqc = nc.declare_dram_parameter("wqc", [HPC * D, 2, DH], e4,
                                    isOutput=False)
    wkc = nc.declare_dram_parameter("wkc", [HPC * D, 2, DH], e4,
                                    isOutput=False)
    ngi = nc.declare_dram_parameter("ngi", [128, 128], f16, isOutput=False)
    out = nc.declare_dram_parameter("out", [N, HPC * N], bf16, isOutput=True)

    nk = D // 128  # 16 contraction chunks
    xm3 = xm.rearrange("(kc p) t -> p kc t", p=128)
    xc4 = xc.rearrange("(kc p) two t -> p kc two t", p=128)
    wm3 = {"q": wqm.rearrange("(kc p) hd -> p kc hd", p=128),
           "k": wkm.rearrange("(kc p) hd -> p kc hd", p=128)}
    wc5 = {"q": wqc.rearrange("(h kc p) two hd -> p h kc two hd",
                              p=128, kc=nk),
           "k": wkc.rearrange("(h kc p) two hd -> p h kc two hd",
                              p=128, kc=nk)}

    with TileContext(nc) as tc, \
         nc.allow_low_precision("fp16/bf16 stages stay within the 2e-2 gate"):
        with tc.tile_pool(name="xT", bufs=1) as xtp, \
             tc.tile_pool(name="w", bufs=2) as wpool, \
             tc.tile_pool(name="qk", bufs=2) as qkp, \
             tc.tile_pool(name="psp", bufs=4, space="PSUM") as psp, \
             tc.tile_pool(name="pss", bufs=2, space="PSUM") as pss, \
             tc.tile_pool(name="ep", bufs=4) as ep, \
             tc.tile_pool(name="outp", bufs=3) as outp:
            cb = xtp.tile([128, 1], f32, name="cb", tag="cb")
            nc.vector.memset(cb[:], -12.0)
            negi = xtp.tile([128, 128], f16, name="negi", tag="negi")
            nc.sync.dma_start(out=negi[:], in_=ngi[:])

            def load_w(h, mat):
                wm = wpool.tile([128, nk * 128], f16, tag=f"wm{mat}")
                nc.sync.dma_start(
                    out=wm[:].rearrange("p (kc hd) -> p kc hd", hd=128),
                    in_=wm3[mat][:, :, ts(h, 128)])
                wc = wpool.tile([128, nk * 2 * 128], e4, tag=f"wc{mat}")
                nc.sync.dma_start(
                    out=wc[:].rearrange("p (kc two hd) -> p kc two hd",
                                        two=2, hd=128),
                    in_=wc5[mat][:, h, :, :, :])
                return wm, wc

            # head 0's weights go first so projection matmuls start within a
            # few us; resident x loads follow, fp16 mains before the e4m3
            # residuals (matching first use).
            wload = {m: load_w(0, m) for m in ("q", "k")}

            xa = xtp.tile([128, nk * N], f16, name="xa", tag="xa")
            xb = xtp.tile([128, nk * 2 * N], e4, name="xb", tag="xb")
            XCH = 2
            for c in range(nk // XCH):
                nc.sync.dma_start(
                    out=xa[:, ds(c * XCH * N, XCH * N)].rearrange(
                        "p (kc t) -> p kc t", t=N),
                    in_=xm3[:, ds(c * XCH, XCH), :])
            for c in range(nk // XCH):
                nc.sync.dma_start(
                    out=xb[:, ds(c * XCH * 2 * N, XCH * 2 * N)].rearrange(
                        "p (kc two t) -> p kc two t", two=2, t=N),
                    in_=xc4[:, ds(c * XCH, XCH), :, :])

            def proj(h, mat):
                """Emit main+corr matmuls and the q1 copyback; return q1,
                qcat and a finalize closure (diag-subtract, residual
                copyback, e4m3 rescale). DoubleRow pairs element 0 with 0 and
                1 with 1, so the residual block goes to slot 0 on the q side
                and slot 1 on the k side."""
                res_blk = 0 if mat == "q" else 1
                wm, wc = wload[mat]

                halves = []
                q1 = qkp.tile([128, N], f16, tag=f"q1{mat}")
                qcat = qkp.tile([128, 2 * N], e4, tag=f"qc{mat}")
                for half in range(2):
                    ps = psp.tile([128, 512], f32, tag="pp")
                    mv = ds(half * 512, 512)
                    for kc in range(nk):
                        nc.tensor.matmul(
                            ps[:], wm[:, ts(kc, 128)],
                            xa[:, ds(kc * N + half * 512, 512)],
                            start=(kc == 0), stop=False)
                    for kc in range(nk):
                        nc.tensor.matmul(
                            ps[:],
                            wc[:, ds(kc * 2 * 128, 2 * 128)].rearrange(
                                "p (two hd) -> p two hd", two=2),
                            xb[:, ds(kc * 2 * N, 2 * N)].rearrange(
                                "p (two t) -> p two t", two=2)
                                [:, :, ds(half * 512, 512)],
                            start=False, stop=(kc == nk - 1),
                            perf_mode=DR, skip_group_check=True)
                    # q1 = 2^6 q (fp16)
                    nc.scalar.activation(q1[:, mv], ps[:], Act.Copy,
                                         bias=0.0, scale=2.0 ** -9)
                    halves.append(ps)

                def fin():
                    for half in range(2):
                        ps = halves[half]
                        mv = ds(half * 512, 512)
                        # ps -= 2^9 * q1  -> ps = 2^15 * (q - fp16(q))
                        nc.tensor.matmul(ps[:], negi[:], q1[:, mv],
                                         start=False, stop=True,
                                         skip_group_check=True)
                        # residual block: 2^10 * q2
                        nc.scalar.activation(
                            qcat[:, ds(res_blk * N + half * 512, 512)],
                            ps[:], Act.Copy, bias=0.0, scale=2.0 ** -5)
                    # main block: 2^2 * q-true = q1 * 2^-4 (ACT; DVE is the
                    # critical engine)
                    nc.scalar.activation(
                        qcat[:, ds((1 - res_blk) * N, N)], q1[:],
                        Act.Copy, bias=0.0, scale=2.0 ** -4)

                return q1, qcat, fin

            def chain(h, qc, sps, eo16):
                # top-2 group selection from the score PSUM (scale-invariant)
                gs = ep.tile([128, G], f32, tag="gs")
                nc.vector.tensor_reduce(
                    gs[:], sps[:].rearrange("p (g j) -> p g j", j=GSIZE),
                    axis=AxX, op=Alu.max)
                m1en = ep.tile([128, 1], f32, tag="m1en")
                nc.vector.tensor_reduce(m1en[:], gs[:], axis=AxX,
                                        op=Alu.max, negate=True)
                sgn1 = ep.tile([128, G], f32, tag="sgn1")
                nc.scalar.activation(sgn1[:], gs[:], Act.Sign,
                                     bias=m1en[:], scale=1.0)
                gs2 = ep.tile([128, G], f32, tag="gs2")
                nc.vector.scalar_tensor_tensor(
                    gs2[:], sgn1[:], 0.0, gs[:], op0=Alu.is_lt, op1=Alu.mult)
                m2en = ep.tile([128, 1], f32, tag="m2en")
                nc.vector.tensor_reduce(m2en[:], gs2[:], axis=AxX,
                                        op=Alu.max, negate=True)
                sgn2 = ep.tile([128, G], f32, tag="sgn2")
                nc.scalar.activation(sgn2[:], gs[:], Act.Sign,
                                     bias=m2en[:], scale=1.0)
                cmp = ep.tile([128, G], f32, tag="cmp")
                nc.vector.tensor_scalar(cmp[:], sgn2[:], 0.0, None,
                                        op0=Alu.is_ge)
                # per-group strips: mask + per-group sums (4x bf16 DVE)
                eo2 = outp.tile([128, N], bf16, tag="eo2")
                gacc = ep.tile([128, G], f32, tag="gacc")
                for g in range(G):
                    nc.vector.tensor_scalar(
                        eo2[:, ts(g, GSIZE)], eo16[:, ts(g, GSIZE)],
                        cmp[:, ds(g, 1)], 0.0, op0=Alu.mult, op1=Alu.add,
                        accum_out=gacc[:, ds(g, 1)])
                ssum = ep.tile([128, 1], f32, tag="ssum")
                nc.vector.tensor_reduce(ssum[:], gacc[:], axis=AxX,
                                        op=Alu.add)
                rc = ep.tile([128, 1], f32, tag="rc")
                nc.vector.reciprocal(rc[:], ssum[:])
                eo3 = outp.tile([128, N], bf16, tag="eo3")
                nc.scalar.activation(eo3[:], eo2[:], Act.Copy,
                                     bias=0.0, scale=rc[:])
                nc.sync.dma_start(
                    out=out[ts(qc, 128), ds(h * N, N)], in_=eo3[:])

            def scores(h, q1, qcat, k1, kcat):
                pend = None
                for qc in range(8):
                    sps = pss.tile([128, N], f32, tag="ss")
                    for half in range(2):
                        mv = ds(half * 512, 512)
                        nc.tensor.matmul(
                            sps[:, mv], q1[:, ts(qc, 128)], k1[:, mv],
                            start=True, stop=False)
                        nc.tensor.matmul(
                            sps[:, mv],
                            qcat[:].rearrange("p (two t) -> p two t", two=2)
                                [:, :, ts(qc, 128)],
                            kcat[:].rearrange("p (two t) -> p two t", two=2)
                                [:, :, ds(half * 512, 512)],
                            start=False, stop=True,
                            perf_mode=DR, skip_group_check=True)
                    eo16 = outp.tile([128, N], bf16, tag="eo16")
                    nc.scalar.activation(eo16[:], sps[:], Act.Exp,
                                         bias=cb[:], scale=2.0 ** -12)
                    if pend is not None:
                        chain(h, *pend)
                    pend = (qc, sps, eo16)
                chain(h, *pend)

            # software pipeline:
            #   [fin_k(h-1)] [scores(h-1)] [Pq(h)] [w-prefetch(h+1)]
            #   [fin_q(h)] [Pk(h)]
            # Scores go at the FRONT of each head block so their matmuls are
            # not queued behind Pq(h) in the PE FIFO — the DVE softmax
            # pipeline starts draining a full proj block earlier. Every
            # proj-PSUM slot's final ops are still emitted before its ring
            # slot is reallocated (fin_k before Pq reuses k's banks, fin_q
            # before Pk reuses q's banks).
            fin_k = None
            prev = None
            for h in range(HPC):
                if fin_k is not None:
                    fin_k()
                if prev is not None:
                    scores(h - 1, *prev)
                q1, qcat, fq = proj(h, "q")
                if h + 1 < HPC:
                    wnext = {m: load_w(h + 1, m) for m in ("q", "k")}
                fq()
                k1, kcat, fk = proj(h, "k")
                fin_k = fk
                prev = (q1, qcat, k1, kcat)
                if h + 1 < HPC:
                    wload = wnext
            fin_k()
            scores(HPC - 1, *prev)

    _install_legalizer(nc)
    return nc


_NC_CACHE = {}


def _get_nc():
    if "nc" not in _NC_CACHE:
        _NC_CACHE["nc"] = _build()
    return _NC_CACHE["nc"]


_E4NP = mybir.dt.np(e4)


def _e4(a):
    return np.clip(np.asarray(a, np.float32), -240.0, 240.0).astype(_E4NP)


def _in_maps(x, Wq, Wk):
    negi = np.zeros((128, 128), np.float16)
    np.fill_diagonal(negi, -512.0)
    maps = []
    for c in range(NCORES):
        b, hh = c // 2, c % 2
        sl = slice(hh * HPC * DH, (hh + 1) * HPC * DH)
        xt = np.ascontiguousarray(x[b].T).astype(np.float32)
        xmain = (xt * 2.0 ** 7).astype(np.float16)
        x2 = xt - xmain.astype(np.float32) / 2.0 ** 7
        xcat = np.stack(
            [_e4(x2 * 2.0 ** 10), _e4(xt * 2.0 ** 1)], axis=1)
        m = {"xm": xmain, "xc": xcat, "ngi": negi}
        for nm, W in (("q", Wq[:, sl] * np.float32(SCALE)), ("k", Wk[:, sl])):
            W = np.ascontiguousarray(W).astype(np.float32)
            wmain = (W * 2.0 ** 8).astype(np.float16)
            w1 = wmain.astype(np.float32) / 2.0 ** 8
            w2 = W - w1
            # per-head-major residuals: [HPC, D, 2, DH] -> [HPC*D, 2, DH]
            w1h = _e4(w1 * 2.0 ** 5).reshape(D, HPC, DH).transpose(1, 0, 2)
            w2h = _e4(w2 * 2.0 ** 14).reshape(D, HPC, DH).transpose(1, 0, 2)
            wcat = np.ascontiguousarray(
                np.stack([w1h, w2h], axis=2)).reshape(HPC * D, 2, DH)
            m[f"w{nm}m"] = wmain
            m[f"w{nm}c"] = wcat
        maps.append(m)
    return maps


def kernel(x, Wq, Wk, **kwargs):
    x = np.asarray(x, dtype=np.float32)
    Wq = np.asarray(Wq, dtype=np.float32)
    Wk = np.asarray(Wk, dtype=np.float32)
    nc = _get_nc()
    res = run_bass_kernel_spmd(nc, _in_maps(x, Wq, Wk),
                               core_ids=list(range(NCORES)))
    full = np.empty((B, N, H, N), dtype=np.float32)
    for c in range(NCORES):
        b, hh = c // 2, c % 2
        full[b, :, hh * HPC:(hh + 1) * HPC, :] = (
            res.results[c]["out"].astype(np.float32).reshape(N, HPC, N))
    return full
